# revision 42
# baseline (speedup 1.0000x reference)
"""Trainium2 Bass kernel for nn_Attention_74268574482673.

Full (unsharded) numpy inputs in -> full [B, D] numpy output.
Strategy: data-parallel over batch B=32 across 8 NeuronCores (4 rows each).
All input layout transforms (transposes / weight fusion / bias folds / bf16
casts) are done host-side as part of sharding; the device kernel runs the
GEMMs and the T=24-step attention recurrence.

Device-side structure per core (B_local=4, col-major layouts, d on
partitions in 4 chunks of 128):
  preamble: co_s[s,e] (bf16, per b) and co_dmT[f,(b,s)] GEMMs from
            transposed inputs (dm_w pre-fused into fc weights on host),
            qm_pre for all T steps; fp32r matmuls (full PE speed, ~tf32).
  scan (24 serial steps): shared rm/rr GEMM (bf16, fast weight load),
            then two independent per-batch-pair chains
            (broadcast-add -> tanh -> logits matvec -> exp/normalize ->
            PE transpose -> weighted sum -> r update) that the Tile
            scheduler interleaves across ACT/DVE/PE.
  epilogue: g = r@rg_w.T + qhs@qg_w.T + biases.
"""

import numpy as np
import ml_dtypes

import concourse.bass as bass
import concourse.bacc as bacc
import concourse.tile as tile
import concourse.mybir as mybir
from concourse.bass_utils import run_bass_kernel_spmd

S, B, T, D = 256, 32, 24, 512
NCORES = 8
BL = B // NCORES  # 4
DC = D // 128     # 4 d-chunks
SC = S // 128     # 2 s-chunks

F32 = mybir.dt.float32
F32R = mybir.dt.float32r
BF16 = mybir.dt.bfloat16
AF = mybir.ActivationFunctionType
ALU = mybir.AluOpType

_compiled = None


def build_kernel():
    nc = bacc.Bacc("TRN2", target_bir_lowering=False, debug=False,
                   num_devices=NCORES)

    def din(name, shape, dt=F32):
        return nc.dram_tensor(name, shape, dt, kind="ExternalInput").ap()

    # ---- DRAM inputs (per-core shards, host-prepped layouts) ----
    xT_ctx_d = din("xT_ctx", [128, DC * BL * S], F32R)   # [d%128, (dc, b, s)]
    xT_img_d = din("xT_img", [128, DC * BL * S], F32R)
    fc1_wT_d = din("fc1_wT", [128, DC * D], F32R)        # [d%128, (dc, e)]
    fc2_wT_d = din("fc2_wT", [128, DC * D], F32R)
    df1_wT_d = din("df1_wT", [128, DC * D], F32R)        # dm_w @ fc1_w, transposed
    df2_wT_d = din("df2_wT", [128, DC * D], F32R)
    qm_wT_d = din("qm_wT", [128, DC * D], F32R)
    rg_wT_d = din("rg_wT", [128, DC * D], F32R)
    qg_wT_d = din("qg_wT", [128, DC * D], F32R)
    w2T_d = din("w2T", [128, DC * 2 * D], BF16)          # [d%128, (dc, 2e)] rm|rr
    ms_wT_d = din("ms_wT", [128, DC * 32], BF16)         # col c*32 live, rest 0
    qT_d = din("qT", [128, DC * T * BL], F32R)           # [d%128, (dc, t, b)]
    qhsT_d = din("qhsT", [128, DC * BL], F32R)           # [d%128, (dc, b)]
    cb_row_d = din("cb_row", [1, D], F32R)               # fc1_b + fc2_b
    dmb_row_d = din("dmb_row", [1, D], F32R)             # dm_w@(fc1_b+fc2_b)+dm_b
    qmb_row_d = din("qmb_row", [1, D], F32R)             # qm_b + rm_b
    rrb_row_d = din("rrb_row", [1, D], BF16)
    gb_row_d = din("gb_row", [1, D], F32R)               # rg_b + qg_b
    msb_col_d = din("msb_col", [128, 1])                 # ms_b broadcast
    ones_row_d = din("ones_row", [1, 1024], F32R)
    onesb_row_d = din("onesb_row", [1, 1024], BF16)
    eye_d = din("eye", [128, 128])

    g_d = nc.dram_tensor("g", [BL, D], F32, kind="ExternalOutput").ap()

    with tile.TileContext(nc) as tc:
        with tc.tile_pool(name="weights", bufs=1) as wpool, \
             tc.tile_pool(name="persist", bufs=1) as ppool, \
             tc.tile_pool(name="smalls", bufs=1) as spool:

            def load(pool, ap_d, name=None, eng=None):
                t = pool.tile(list(ap_d.shape), ap_d.dtype, name=name)
                (eng or nc.scalar).dma_start(t[:], ap_d[:])
                return t

            fc1_wT = load(wpool, fc1_wT_d, name="fc1_wT")
            fc2_wT = load(wpool, fc2_wT_d, name="fc2_wT")
            df1_wT = load(wpool, df1_wT_d, name="df1_wT")
            df2_wT = load(wpool, df2_wT_d, name="df2_wT")
            qm_wT = load(wpool, qm_wT_d, name="qm_wT")
            w2T = load(wpool, w2T_d, name="w2T")
            ms_wT = load(wpool, ms_wT_d, name="ms_wT")
            qT = load(spool, qT_d, name="qT")
            qhsT = load(spool, qhsT_d, name="qhsT")
            cb_row = load(spool, cb_row_d, name="cb_row")
            dmb_row = load(spool, dmb_row_d, name="dmb_row")
            qmb_row = load(spool, qmb_row_d, name="qmb_row")
            rrb_row = load(spool, rrb_row_d, name="rrb_row")
            gb_row = load(spool, gb_row_d, name="gb_row")
            msb_col = load(spool, msb_col_d, name="msb_col")
            ones_row = load(spool, ones_row_d, name="ones_row")
            onesb_row = load(spool, onesb_row_d, name="onesb_row")
            eye = load(spool, eye_d, name="eye")

            co_dmT = ppool.tile([128, DC * BL * S], BF16, name="co_dmT")
            co_s = ppool.tile([128, SC * BL * D], BF16, name="co_s")
            qm_preT = ppool.tile([128, DC * T * BL], F32, name="qm_preT")

            with tc.tile_pool(name="xT", bufs=1) as xpool, \
                 tc.tile_pool(name="pre_ps", bufs=4, space="PSUM") as pps:
                # chunked loads so the first GEMMs start before the full
                # 2MB tensor lands
                xT_ctx = xpool.tile(list(xT_ctx_d.shape), xT_ctx_d.dtype,
                                    name="xT_ctx")
                xT_img = xpool.tile(list(xT_img_d.shape), xT_img_d.dtype,
                                    name="xT_img")
                for c in range(DC):
                    sl = slice(c * BL * S, (c + 1) * BL * S)
                    nc.sync.dma_start(xT_ctx[:, sl], xT_ctx_d[:, sl])
                    nc.sync.dma_start(xT_img[:, sl], xT_img_d[:, sl])

                # ---- P4: co_dmT[f,(b,s)] = df1^T·xc + df2^T·xi + dmb ----
                for j in range(DC):  # f-chunk
                    for h in range(2):  # halves of (b,s)=1024
                        ps = pps.tile([128, 512], F32, tag="pre", name="ps_dm")
                        first = True
                        for c in range(DC):
                            l1 = df1_wT[:, c * D + j * 128:c * D + (j + 1) * 128]
                            l2 = df2_wT[:, c * D + j * 128:c * D + (j + 1) * 128]
                            rx = xT_ctx[:, c * 1024 + h * 512:c * 1024 + (h + 1) * 512]
                            ri = xT_img[:, c * 1024 + h * 512:c * 1024 + (h + 1) * 512]
                            nc.tensor.matmul(ps[:], l1, rx, start=first, stop=False)
                            first = False
                            nc.tensor.matmul(ps[:], l2, ri, start=False, stop=False)
                        nc.tensor.matmul(
                            ps[:], dmb_row[0:1, j * 128:(j + 1) * 128],
                            ones_row[0:1, 0:512], start=False, stop=True)
                        dst = co_dmT[:, j * 1024 + h * 512:j * 1024 + (h + 1) * 512]
                        if (j + h) % 2 == 0:
                            nc.vector.tensor_copy(dst, ps[:])
                        else:
                            nc.scalar.copy(dst, ps[:])

                # ---- P5: qm_preT[e,(t,b)] = qm^T·qT + (qm_b+rm_b) ----
                for j in range(DC):
                    ps = pps.tile([128, T * BL], F32, tag="preq", name="ps_qm")
                    first = True
                    for c in range(DC):
                        lw = qm_wT[:, c * D + j * 128:c * D + (j + 1) * 128]
                        rq = qT[:, c * T * BL:(c + 1) * T * BL]
                        nc.tensor.matmul(ps[:], lw, rq, start=first, stop=False)
                        first = False
                    nc.tensor.matmul(
                        ps[:], qmb_row[0:1, j * 128:(j + 1) * 128],
                        ones_row[0:1, 0:T * BL], start=False, stop=True)
                    nc.vector.tensor_copy(
                        qm_preT[:, j * T * BL:(j + 1) * T * BL], ps[:])

                # ---- P3: co_s[(sc,b)] [s%128, e] = x^T-slices · fc_wT + cb ----
                for b in range(BL):
                    for sc in range(SC):
                        ps = pps.tile([128, 512], F32, tag="pre", name="ps_cos")
                        first = True
                        for c in range(DC):
                            off = c * 1024 + b * 256 + sc * 128
                            lx = xT_ctx[:, off:off + 128]
                            li = xT_img[:, off:off + 128]
                            r1 = fc1_wT[:, c * D:(c + 1) * D]
                            r2 = fc2_wT[:, c * D:(c + 1) * D]
                            nc.tensor.matmul(ps[:], lx, r1, start=first, stop=False)
                            first = False
                            nc.tensor.matmul(ps[:], li, r2, start=False, stop=False)
                        nc.tensor.matmul(
                            ps[:], ones_row[0:1, 0:128], cb_row[0:1, :],
                            start=False, stop=True)
                        dst = co_s[:, (sc * BL + b) * D:(sc * BL + b + 1) * D]
                        if (b + sc) % 2 == 0:
                            nc.vector.tensor_copy(dst, ps[:])
                        else:
                            nc.scalar.copy(dst, ps[:])

            # ---- scan over T steps: shared S1, then 2 per-pair chains ----
            with tc.tile_pool(name="rT", bufs=2) as rpool, \
                 tc.tile_pool(name="addp", bufs=1, space="PSUM") as addp, \
                 tc.tile_pool(name="logitp", bufs=1, space="PSUM") as logitp, \
                 tc.tile_pool(name="wtp", bufs=1, space="PSUM") as wtp, \
                 tc.tile_pool(name="rnewp", bufs=1, space="PSUM") as rnewp, \
                 tc.tile_pool(name="mpre", bufs=6) as mpool, \
                 tc.tile_pool(name="mtan", bufs=10) as m2pool, \
                 tc.tile_pool(name="stepsm", bufs=4) as smpool:

                rT_bf = []
                for p_ in range(2):
                    rt = rpool.tile([128, DC * 2], BF16, tag=f"rT{p_}",
                                    name="rT0")
                    nc.vector.memset(rt[:], 0.0)
                    rT_bf.append(rt)
                rT_f32_final = ppool.tile([128, DC * BL], F32R, name="rT_f32")

                qv = qm_preT[:].rearrange("p (c n) -> p c n", c=DC)

                for t in range(T):
                    rT_next = []
                    for p_ in range(2):
                        rt = rpool.tile([128, DC * 2], BF16, tag=f"rT{p_}",
                                        name="rT_bf")
                        rT_next.append(rt)

                    for p in range(2):  # fully decoupled per-pair chains
                        # S1_p: psum_addT [128, 16] cols j*2+k;
                        # j 0..3 rm-echunks, 4..7 rr-echunks
                        ps_add = addp.tile([128, 16], F32, tag=f"add{p}",
                                           name="ps_add")
                        first = True
                        for j in range(8):
                            for c in range(DC):
                                lw = w2T[:, c * 2 * D + j * 128:
                                         c * 2 * D + (j + 1) * 128]
                                rr_ = rT_bf[p][:, c * 2:(c + 1) * 2]
                                nc.tensor.matmul(
                                    ps_add[:, j * 2:(j + 1) * 2], lw, rr_,
                                    start=first, stop=False)
                                first = False
                        for j in range(4):  # rr bias via K=1 (bf16)
                            nc.tensor.matmul(
                                ps_add[:, (4 + j) * 2:(5 + j) * 2],
                                rrb_row[0:1, j * 128:(j + 1) * 128],
                                onesb_row[0:1, 0:2], start=False,
                                stop=(j == 3))
                        pa = ps_add[:].rearrange("q (j k) -> q j k", j=8)

                        # addT_p = ps_add rm-cols + qm_preT[t, pair]
                        addT = smpool.tile([128, 2 * DC], F32, tag=f"addT{p}",
                                           name="addT")
                        av = addT[:].rearrange("q (c k) -> q c k", c=DC)
                        nc.vector.tensor_tensor(
                            av[:, :, :], pa[:, 0:DC, :],
                            qv[:, :, t * BL + p * 2:t * BL + (p + 1) * 2],
                            op=ALU.add)
                        # tanh_rr_p (ACT, psum -> sbuf)
                        tanh_rr = smpool.tile([128, 2 * DC], F32, tag=f"trr{p}",
                                              name="tanh_rr")
                        tv = tanh_rr[:].rearrange("q (c k) -> q c k", c=DC)
                        nc.scalar.activation(tv[:, :, :], pa[:, DC:2 * DC, :],
                                             AF.Tanh)

                        # Per-c: adds (DVE, bf16 4x) -> tanh (ACT) -> logits
                        # (PE) immediately, so ACT and PE interleave instead
                        # of PE waiting for the whole tanh.
                        ps_log = logitp.tile([64, 256], F32, tag=f"log{p}",
                                             name="ps_log")
                        m_tiles = []
                        for ch in range(2):  # c-pair halves
                            mp = mpool.tile([128, 1024], BF16, tag="mpre",
                                            name="m_pre")
                            for ck in range(2):
                                c = ch * 2 + ck
                                for k in range(2):
                                    b = p * 2 + k
                                    src = co_dmT[:, c * 1024 + b * 256:
                                                 c * 1024 + (b + 1) * 256]
                                    nc.vector.tensor_scalar_add(
                                        mp[:, ck * 512 + k * 256:
                                           ck * 512 + (k + 1) * 256], src,
                                        addT[:, c * 2 + k:c * 2 + k + 1])
                            mt = m2pool.tile([128, 1024], BF16, tag="m",
                                             name="m_t")
                            nc.scalar.activation(mt[:], mp[:], AF.Tanh)
                            m_tiles.append(mt)
                            # k=0 logits bracket interleaves with tanh
                            for ck in range(2):
                                c = ch * 2 + ck
                                nc.tensor.matmul(
                                    ps_log[0:32, :],
                                    ms_wT[:, c * 32:(c + 1) * 32],
                                    mt[:, ck * 512:ck * 512 + 256],
                                    start=(c == 0), stop=(c == DC - 1),
                                    tile_position=(0, 0))
                        for c in range(DC):  # k=1 bracket
                            nc.tensor.matmul(
                                ps_log[32:64, :],
                                ms_wT[:, c * 32:(c + 1) * 32],
                                m_tiles[c // 2][:, (c % 2) * 512 + 256:
                                                (c % 2) * 512 + 512],
                                start=(c == 0), stop=(c == DC - 1),
                                tile_position=(0, 32))

                        # softmax (unnormalized exp -> sum -> recip -> scale)
                        w_raw = smpool.tile([64, 256], F32, tag=f"wraw{p}",
                                            name="w_raw")
                        sums = smpool.tile([64, 1], F32, tag=f"sums{p}",
                                           name="sums")
                        nc.scalar.activation(w_raw[:], ps_log[:], AF.Exp,
                                             bias=msb_col[0:64, 0:1],
                                             accum_out=sums[:])
                        recip = smpool.tile([64, 1], F32, tag=f"recip{p}",
                                            name="recip")
                        nc.vector.reciprocal(recip[:], sums[:])
                        w_norm = smpool.tile([64, 256], F32, tag=f"wnorm{p}",
                                             name="w_norm")
                        nc.vector.tensor_scalar_mul(w_norm[:], w_raw[:],
                                                    recip[:, 0:1])

                        # transpose w_norm -> wT bf16 (PE + DVE evac)
                        wT_bf = smpool.tile([128, SC * 64], BF16, tag=f"wT{p}",
                                            name="wT_bf")
                        ps_wt = wtp.tile([128, 128], F32, tag=f"wt{p}",
                                         name="ps_wt")
                        for sc in range(SC):
                            nc.tensor.transpose(
                                ps_wt[:, sc * 64:(sc + 1) * 64],
                                w_norm[:, sc * 128:(sc + 1) * 128],
                                eye[0:64, 0:64])
                            nc.vector.tensor_copy(
                                wT_bf[:, sc * 64:(sc + 1) * 64],
                                ps_wt[:, sc * 64:(sc + 1) * 64])

                        # weighted sum -> ps_r [128, 8] cols c*2+k
                        ps_r = rnewp.tile([128, 2 * DC], F32, tag=f"rnew{p}",
                                          name="ps_r")
                        first_w = True
                        for k in range(2):
                            b = p * 2 + k
                            for c in range(DC):
                                for sc in range(SC):
                                    lco = co_s[:, (sc * BL + b) * D + c * 128:
                                               (sc * BL + b) * D + (c + 1) * 128]
                                    rw = wT_bf[:, sc * 64 + k * 32:
                                               sc * 64 + k * 32 + 1]
                                    nc.tensor.matmul(
                                        ps_r[:, c * 2 + k:c * 2 + k + 1],
                                        lco, rw, start=first_w,
                                        stop=(k == 1 and c == DC - 1
                                              and sc == SC - 1))
                                    first_w = False

                        # r update: per-pair rT tile = ps_r + tanh_rr
                        nc.vector.tensor_tensor(
                            rT_next[p][:].rearrange("q (c k) -> q c k", c=DC),
                            ps_r[:].rearrange("q (c k) -> q c k", c=DC),
                            tv[:, :, :], op=ALU.add)
                        if t == T - 1:
                            rfv = rT_f32_final[:].rearrange(
                                "q (c b) -> q c b", c=DC)
                            nc.vector.tensor_tensor(
                                rfv[:, :, p * 2:(p + 1) * 2],
                                ps_r[:].rearrange("q (c k) -> q c k", c=DC),
                                tv[:, :, :], op=ALU.add)
                    rT_bf = rT_next

            # ---- epilogue: g = r·rg^T + qhs·qg^T + (rg_b+qg_b) ----
            with tc.tile_pool(name="gp", bufs=1, space="PSUM") as gp, \
                 tc.tile_pool(name="gout", bufs=1) as gop:
                rg_wT = load(wpool, rg_wT_d, name="rg_wT")
                qg_wT = load(wpool, qg_wT_d, name="qg_wT")
                rT_r = rT_f32_final[:]
                ps_g = gp.tile([BL, D], F32, name="ps_g")
                first = True
                for c in range(DC):
                    nc.tensor.matmul(ps_g[:], rT_r[:, c * BL:(c + 1) * BL],
                                     rg_wT[:, c * D:(c + 1) * D],
                                     start=first, stop=False)
                    first = False
                    nc.tensor.matmul(ps_g[:], qhsT[:, c * BL:(c + 1) * BL],
                                     qg_wT[:, c * D:(c + 1) * D],
                                     start=False, stop=False)
                nc.tensor.matmul(ps_g[:], ones_row[0:1, 0:BL],
                                 gb_row[0:1, :], start=False, stop=True)
                g_sb = gop.tile([BL, D], F32, name="g_sb")
                nc.scalar.copy(g_sb[:], ps_g[:])
                nc.sync.dma_start(g_d[:], g_sb[:])

    nc.compile()
    return nc


def _ms_lay(ms_w):
    # [1, D] -> [128, (dc, 32)]: column c*32 holds ms_w chunk c, rest zero
    out = np.zeros((128, DC * 32), np.float32)
    for c in range(DC):
        out[:, c * 32] = ms_w[0, c * 128:(c + 1) * 128]
    return out


def prep_inputs(inputs):
    """Host-side sharding + layout prep. Returns list of per-core in_maps."""
    f32 = np.float32
    bf = ml_dtypes.bfloat16
    ctx = np.asarray(inputs['context_output'], f32)
    img = np.asarray(inputs['image_output'], f32)
    q = np.asarray(inputs['question_output'], f32)
    qhs = np.asarray(inputs['question_hidden_state'], f32)

    def wT_lay(w):  # [e,d] -> [128, (dc, e)]
        wT = np.ascontiguousarray(np.asarray(w, f32).T)  # [d, e]
        return np.ascontiguousarray(
            wT.reshape(DC, 128, D).transpose(1, 0, 2).reshape(128, DC * D))

    def dT_lay(x2d):  # [d, n] -> [128, (dc, n)]
        n = x2d.shape[1]
        return np.ascontiguousarray(
            x2d.reshape(DC, 128, n).transpose(1, 0, 2).reshape(128, DC * n))

    df1 = np.asarray(inputs['dm_w'], f32) @ np.asarray(inputs['fc1_w'], f32)
    df2 = np.asarray(inputs['dm_w'], f32) @ np.asarray(inputs['fc2_w'], f32)
    cb = np.asarray(inputs['fc1_b'], f32) + np.asarray(inputs['fc2_b'], f32)
    dmb = np.asarray(inputs['dm_w'], f32) @ cb + np.asarray(inputs['dm_b'], f32)

    shared = {
        'fc1_wT': wT_lay(inputs['fc1_w']),
        'fc2_wT': wT_lay(inputs['fc2_w']),
        'df1_wT': wT_lay(df1),
        'df2_wT': wT_lay(df2),
        'qm_wT': wT_lay(inputs['qm_w']),
        'rg_wT': wT_lay(inputs['rg_w']),
        'qg_wT': wT_lay(inputs['qg_w']),
        'w2T': np.ascontiguousarray(
            np.concatenate([np.asarray(inputs['rm_w'], f32).T,
                            np.asarray(inputs['rr_w'], f32).T], axis=1)
            .reshape(DC, 128, 2 * D).transpose(1, 0, 2)
            .reshape(128, DC * 2 * D)).astype(bf),
        'ms_wT': _ms_lay(np.asarray(inputs['ms_w'], f32)).astype(bf),
        'cb_row': cb.reshape(1, D),
        'dmb_row': dmb.reshape(1, D).astype(f32),
        'qmb_row': (np.asarray(inputs['qm_b'], f32)
                    + np.asarray(inputs['rm_b'], f32)).reshape(1, D),
        'rrb_row': np.asarray(inputs['rr_b'], f32).reshape(1, D).astype(bf),
        'gb_row': (np.asarray(inputs['rg_b'], f32)
                   + np.asarray(inputs['qg_b'], f32)).reshape(1, D),
        'msb_col': np.full((128, 1), float(np.asarray(inputs['ms_b'])[0]), f32),
        'ones_row': np.ones((1, 1024), f32),
        'onesb_row': np.ones((1, 1024), bf),
        'eye': np.eye(128, dtype=f32),
    }

    in_maps = []
    for i in range(NCORES):
        bsl = slice(i * BL, (i + 1) * BL)
        m = dict(shared)
        m['xT_ctx'] = dT_lay(ctx[:, bsl, :].transpose(2, 1, 0).reshape(D, BL * S))
        m['xT_img'] = dT_lay(img[:, bsl, :].transpose(2, 1, 0).reshape(D, BL * S))
        m['qT'] = dT_lay(q[:, bsl, :].transpose(2, 0, 1).reshape(D, T * BL))
        m['qhsT'] = dT_lay(np.ascontiguousarray(qhs[bsl].T))
        in_maps.append(m)
    return in_maps


def kernel(**inputs):
    global _compiled
    if _compiled is None:
        _compiled = build_kernel()
    in_maps = prep_inputs(inputs)
    last_err = None
    for attempt in range(3):
        try:
            res = run_bass_kernel_spmd(_compiled, in_maps, list(range(NCORES)))
            return np.concatenate(
                [res.results[i]['g'] for i in range(NCORES)], axis=0)
        except Exception as e:  # transient device wedge -> retry
            last_err = e
            import os as _os
            import time as _time
            _os.environ["NEURON_RT_RESET_CORES"] = "1"
            _time.sleep(5 * (attempt + 1))
    raise last_err
